# revision 32
# baseline (speedup 1.0000x reference)
"""Trainium2 Bass kernel for nn_Capa_Harmonica_1 (segment_reduce).

Math: the reference's complex harmonic conv + aliasing fold collapses exactly.
The conv kernel is W[o,c,t] = |A|e^{i(beta + w t)} with w = 2*pi*m/N and
w*ker = pi, so the alternating-sign aliasing fold telescopes the windowed conv
into the full modulated sum. End to end:

    Q[b,c]  = sum_u Z[b,c,u] e^{i w u}              (Z = z_real + i z_imag)
    G[b,o]  = sum_c |A[o,c]| e^{i beta[o,c]} Q[b,c]
    gate    = sigmoid(|G|+bias) / (|G|+1e-5)
    out[b,o,mu] = Re/Im( gate * G[b,o] e^{-i w mu} )

For the harness inputs min(|G|+bias) ~ 15, so sigmoid == 1 to 2.6e-7 and the
1e-5 eps is a ~7e-7 relative perturbation; the device program uses
gate = 1/|G| and a host-side guard falls back to the exact host reference
whenever the saturation condition doesn't hold.

Sharding: 8 cores = batch (4) x c_out-half (2). Per core, one fp16 input DMA
carries [zr | zi | cosm | -sinm | sinm | cosm] so the whole modulated
reduction is TWO fused DVE multiply-accumulate passes over paired columns
(complex real/imag parts in one op each). The 16-segment fold and the channel
contraction fuse into one pair of fp32 PE matmuls against a host-expanded
W0 (w0exp[p,o] = W0[p//16, o]). The gate is a 6-op fp32 chain, and both
(128 x 512) output slabs are single-pass fp16 PE matmuls coef^T @ [cos; sin];
outputs store as fp16 (host upcasts) and the HBM writes duplicate the
512-period via stride-0 source APs.
"""

import numpy as np

_KB, _COUT, _CIN, _N = 4, 64, 8, 4096
_OC = _COUT // 2  # out channels per core
_NCORES = 8

_cache = {}


def _build_program(mval: int):
    import concourse.bacc as bacc
    import concourse.bass as bass
    import concourse.mybir as mybir
    import concourse.tile as tile

    dt = mybir.dt
    AF = mybir.ActivationFunctionType
    ALU = mybir.AluOpType
    f32 = dt.float32
    f16 = dt.float16

    # skip the const-AP memsets + all-engine barrier Bass.__init__ emits
    # (~1us of preamble); every activation bias below is an explicit AP so
    # the pre-initialized const tensors are never read
    _orig_barrier = bass.Bass.all_engine_barrier
    _orig_memset = bass.BassSharedVectorInterface.memset
    bass.Bass.all_engine_barrier = lambda self: None
    bass.BassSharedVectorInterface.memset = lambda self, ap, c: None
    try:
        nc = bacc.Bacc(
            "TRN2", target_bir_lowering=False, debug=False, num_devices=_NCORES
        )
    finally:
        bass.Bass.all_engine_barrier = _orig_barrier
        bass.BassSharedVectorInterface.memset = _orig_memset

    zz_d = nc.dram_tensor("zz", [128, 512], f16, kind="ExternalInput")
    tb_d = nc.dram_tensor("tbl", [128, 768], f16, kind="ExternalInput")
    w0_d = nc.dram_tensor("w0e", [128, 64], f32, kind="ExternalInput")
    bs_d = nc.dram_tensor("bas", [32, 640], f16, kind="ExternalInput")
    or_d = nc.dram_tensor("o_r", [128, 512], f16, kind="ExternalOutput")
    oi_d = nc.dram_tensor("o_i", [128, 512], f16, kind="ExternalOutput")

    with tile.TileContext(nc) as tc:
        with (
            tc.tile_pool(name="sb", bufs=1) as sb,
            tc.tile_pool(name="ps", bufs=1, space="PSUM") as ps,
        ):
            # z pair on the SP ring, tables first on the ACT ring — the two
            # big transfers and their completion semaphores overlap
            zz = sb.tile([128, 512], f16)
            nc.sync.dma_start(zz[:], zz_d[:])
            tbl = sb.tile([128, 768], f16)
            nc.scalar.dma_start(tbl[:], tb_d[:])
            bs = sb.tile([32, 640], f16)
            nc.scalar.dma_start(bs[:], bs_d[:])
            w0e = sb.tile([128, 64], f32)
            nc.scalar.dma_start(w0e[:], w0_d[:])

            basis2 = bs[0:2, 0:512]   # rows cos(w f) | sin(w f)
            rep16 = bs[0:32, 512:640]  # rep[c,p] = (p//4 == c)

            zero32 = sb.tile([32, 1], f32)
            nc.vector.memset(zero32[:], 0.0)

            # whole modulated reduction in two fused DVE passes over the
            # complex pair [zi | zr] against overlapping table windows
            # [-sinm | cosm] and [cosm | sinm]:
            # qq[:,1] = sum_f zr*cosm - zi*sinm   (per-partition Qr partials)
            # qq[:,2] = sum_f zr*sinm + zi*cosm   (per-partition Qi partials)
            qq = sb.tile([128, 3], f32)
            scr = sb.tile([128, 512], f32)
            scr2 = sb.tile([128, 512], f32)
            nc.vector.scalar_tensor_tensor(
                scr[:], zz[:], 1.0, tbl[:, 0:512],
                ALU.bypass, ALU.mult, accum_out=qq[:, 1:2],
            )
            nc.vector.scalar_tensor_tensor(
                scr2[:], zz[:], 1.0, tbl[:, 256:768],
                ALU.bypass, ALU.mult, accum_out=qq[:, 2:3],
            )
            nc.vector.tensor_scalar_mul(qq[:, 0:1], qq[:, 2:3], -1.0)

            # segment fold + channel contraction in one fp32 matmul pair:
            # G[o, :] = sum_p w0r_exp[p,o]*[Qr,Qi] + w0i_exp[p,o]*[-Qi,Qr]
            g_ps = ps.tile([_OC, 2], f32, tag="small", bufs=3)
            nc.tensor.matmul(g_ps[:], w0e[:, 0:32], qq[:, 1:3], start=True, stop=False)
            nc.tensor.matmul(g_ps[:], w0e[:, 32:64], qq[:, 0:2], start=False, stop=True)

            # gate = 1/|G| (sigmoid saturated, eps dropped; host guard)
            g_sb = sb.tile([_OC, 2], f32)
            nc.vector.tensor_copy(g_sb[:], g_ps[:])
            g_scr = sb.tile([_OC, 2], f32)
            magsq = sb.tile([_OC, 1], f32)
            nc.vector.scalar_tensor_tensor(
                g_scr[:], g_sb[:], 1.0, g_ps[:], ALU.bypass, ALU.mult,
                accum_out=magsq[:],
            )
            mag = sb.tile([_OC, 1], f32)
            nc.scalar.activation(mag[:], magsq[:], AF.Sqrt, bias=zero32[:])
            gate = sb.tile([_OC, 1], f32)
            nc.vector.reciprocal(gate[:], mag[:])
            h3 = sb.tile([_OC, 3], f16)
            nc.vector.tensor_scalar(h3[:, 0:2], g_sb[:, 0:2], gate[:], None, ALU.mult)
            nc.vector.tensor_scalar(
                h3[:, 2:3], g_sb[:, 0:1], gate[:], -1.0, ALU.mult, ALU.mult
            )

            # coef pairs [Grg; Gig] and [Gig; -Grg], 32ch -> 128 cols, both in
            # one PSUM tile -> one fp16 cast
            coef_ps = ps.tile([2, 256], f32, tag="small", bufs=3)
            nc.tensor.matmul(coef_ps[:, 0:128], h3[:, 0:2], rep16, start=True, stop=True)
            nc.tensor.matmul(coef_ps[:, 128:256], h3[:, 1:3], rep16, start=True, stop=True)
            coef = sb.tile([2, 256], f16)
            nc.vector.tensor_copy(coef[:], coef_ps[:])

            # out_r = Grg*cos + Gig*sin, out_i = Gig*cos - Grg*sin as
            # single-pass fp16 PE matmuls against the one-period basis
            outr_ps = ps.tile([128, 512], f32, tag="obr", bufs=1)
            nc.tensor.matmul(outr_ps[:], coef[:, 0:128], basis2, start=True, stop=True)
            outi_ps = ps.tile([128, 512], f32, tag="obi", bufs=1)
            nc.tensor.matmul(outi_ps[:], coef[:, 128:256], basis2, start=True, stop=True)

            outr_sb = sb.tile([128, 512], f16)
            nc.vector.tensor_copy(outr_sb[:], outr_ps[:])
            outi_sb = sb.tile([128, 512], f16)
            nc.scalar.copy(outi_sb[:], outi_ps[:])

            # one period per output; the host assembles the exact periodic
            # extension when unsharding
            nc.sync.dma_start(or_d[:], outr_sb[:])
            nc.scalar.dma_start(oi_d[:], outi_sb[:])

    nc.compile()
    return nc


def _host_tables(mval: int):
    w = 2.0 * np.pi * mval / _N
    p = np.arange(128)[:, None]
    f = np.arange(256)[None, :]
    u = (p % 16) * 256 + f
    cosm = np.cos(w * u).astype(np.float16)          # (128, 256)
    sinm = np.sin(w * u).astype(np.float16)
    # pairs with [zi | zr]: window [0:512] gives Qr, [256:768] gives Qi
    tbl = np.concatenate([-sinm, cosm, sinm], axis=1)
    fb = np.arange(512)
    basis2 = np.stack([np.cos(w * fb), np.sin(w * fb)]).astype(np.float16)
    rep = (np.arange(32)[:, None] == np.arange(128)[None, :] // 4)
    bas = np.zeros((32, 640), np.float16)
    bas[0:2, 0:512] = basis2
    bas[:, 512:640] = rep.astype(np.float16)
    return tbl, bas


def _host_reference(z_real, z_imag, A, beta, bias, m):
    # exact analytic fallback (float64) for inputs outside the fast path
    w = 2.0 * np.pi * m / _N
    u = np.arange(_N)
    Z = z_real.astype(np.float64) + 1j * z_imag.astype(np.float64)
    Q = (Z * np.exp(1j * w * u)).sum(-1)
    W0 = np.abs(A[:, :, 0]).astype(np.float64) * np.exp(1j * beta[:, :, 0].astype(np.float64))
    G = Q @ W0.T
    magG = np.abs(G)
    gate = 1.0 / (1.0 + np.exp(-(magG + bias[None, :, 0]))) / (magG + 1e-5)
    H = gate * G
    S = H[:, :, None] * np.exp(-1j * w * u)[None, None, :]
    return S.real.astype(np.float32), S.imag.astype(np.float32)


def _run(z_real, z_imag, A, beta, bias, m, trace=False, **spmd_kwargs):
    from concourse.bass_utils import run_bass_kernel_spmd

    mval = int(m)
    z_real = np.ascontiguousarray(z_real, dtype=np.float32)
    z_imag = np.ascontiguousarray(z_imag, dtype=np.float32)
    A = np.ascontiguousarray(A, dtype=np.float32)
    beta = np.ascontiguousarray(beta, dtype=np.float32)
    bias = np.ascontiguousarray(bias, dtype=np.float32)

    ok = (
        mval % 8 == 0 and mval != 0 and _N % (2 * abs(mval)) == 0
        and z_real.shape == (_KB, _CIN, _N)
    )
    if ok:
        # host guard for the saturated-sigmoid/eps-free fp16 device gate
        w = 2.0 * np.pi * mval / _N
        e = np.exp(1j * w * np.arange(_N))
        Q = (z_real.astype(np.float64) + 1j * z_imag.astype(np.float64)) @ e
        W0 = np.abs(A[:, :, 0]).astype(np.float64) * np.exp(
            1j * beta[:, :, 0].astype(np.float64)
        )
        magG = np.abs(Q @ W0.T)
        ok = (magG + bias[None, :, 0]).min() > 12.0 and magG.min() > 1e-2

    if not ok:
        return _host_reference(z_real, z_imag, A, beta, bias, mval) + (None,)

    if mval not in _cache:
        _cache[mval] = (_build_program(mval), _host_tables(mval))
    nc, (tbl, bas) = _cache[mval]

    w0 = np.abs(A[:, :, 0]) * np.exp(1j * beta[:, :, 0].astype(np.float64))
    in_maps = []
    for core in range(_NCORES):
        b, h = core // 2, core % 2
        o0, o1 = h * _OC, (h + 1) * _OC
        zz = np.concatenate(
            [
                z_imag[b].reshape(128, 256).astype(np.float16),
                z_real[b].reshape(128, 256).astype(np.float16),
            ],
            axis=1,
        )
        w0e = np.concatenate(
            [
                np.repeat(w0[o0:o1].T.real, 16, axis=0),
                np.repeat(w0[o0:o1].T.imag, 16, axis=0),
            ],
            axis=1,
        ).astype(np.float32)
        in_maps.append(
            {"zz": np.ascontiguousarray(zz), "tbl": tbl, "w0e": w0e, "bas": bas}
        )

    res = run_bass_kernel_spmd(
        nc, in_maps, core_ids=list(range(_NCORES)), trace=trace, **spmd_kwargs
    )

    out_r = np.empty((_KB, _COUT, _N), np.float32)
    out_i = np.empty((_KB, _COUT, _N), np.float32)
    for core in range(_NCORES):
        b, h = core // 2, core % 2
        o0, o1 = h * _OC, (h + 1) * _OC
        out_r[b, o0:o1] = np.tile(
            res.results[core]["o_r"].astype(np.float32), (1, 2)
        ).reshape(_OC, _N)
        out_i[b, o0:o1] = np.tile(
            res.results[core]["o_i"].astype(np.float32), (1, 2)
        ).reshape(_OC, _N)
    return out_r, out_i, res


def kernel(z_real, z_imag, A, beta, bias, m):
    out_r, out_i, _ = _run(z_real, z_imag, A, beta, bias, m)
    return out_r, out_i
